# revision 2
# baseline (speedup 1.0000x reference)
"""Trainium2 Bass kernel for nn_MindPalaceRouter (v6).

Computation (reference):
    ctx  = mean_T(x) @ Wc.T + bc                      [B, d]
    warp = (ctx @ Ww.T + bw).reshape(B, n, n) * 0.1
    adj  = softmax(adjacency + warp, axis=-1)
    raw  = ctx @ summaries.T                          [B, n]
    gate = sigmoid((ctx @ Wg.T + bg + adj @ raw) / 2) [B, n]

Strategy: data-parallel over B (4 samples/core, no collectives).  Wc is
folded into every downstream projection on the host (weights-only
reparametrization), so ctx is never materialized:
    warp path:  Wcw = Ww @ Wc, factored by SVD to rank 256 (keeps
                absmax-rel error ~3e-5, three orders under the 2e-2
                gate) and run as two thin fp8 matmuls, the second in
                DoubleRow mode.
    raw  path:  Wcs = summaries @ Wc  (bf16)
    gate path:  Wcg = Wg @ Wc         (bf16, computed transposed)
x ships fp8 with 16KB-contiguous per-partition runs; the T-sums
pipeline across samples in per-sample single-bank PSUM accumulators.
The adjacency+bias row is preloaded into each warp PSUM tile by the
(otherwise idle) DVE in f32.  The softmax message tail runs in a
transposed [m, (b, n)] layout reached via a DRAM bounce, so its
reduces are 32-64-partition-wide; the finals stay transposed and the
host transposes the [64, 4] per-core output back.
"""

import sys

if "/opt/trn_rl_repo" not in sys.path:
    sys.path.insert(0, "/opt/trn_rl_repo")

import numpy as np

N_CORES = 8
B, T, D, NR = 32, 2048, 1024, 64
NN = NR * NR  # 4096
BSH = B // N_CORES  # 4 samples per core
DCH = D // 128  # 8 d-chunks
SCH = 16  # T-rows per partition per x tile (16KB contiguous runs)
RNK = 256  # Wcw SVD truncation rank
VS = 64.0  # fp8 V-factor pre-scale (V entries ~1/32)
TS = 2.0  # extra scale on the fp8 intermediate copy
US = 64.0  # fp8 U-factor pre-scale (U*S entries ~1/64)
CSC = VS * TS * US  # 8192: total scale compensated in the Exp

_cache = {}


def _build_nc():
    import concourse.bass as bass
    import concourse.tile as tile
    from concourse import bacc, mybir

    f32 = mybir.dt.float32
    bf16 = mybir.dt.bfloat16
    f8 = mybir.dt.float8e4
    AF = mybir.ActivationFunctionType
    DR = mybir.MatmulPerfMode.DoubleRow

    nc = bacc.Bacc(
        "TRN2",
        target_bir_lowering=False,
        debug=False,
        enable_asserts=True,
        num_devices=N_CORES,
    )

    xs_d = nc.dram_tensor("xs", [BSH, T, D], f8, kind="ExternalInput")
    # V-factor: [d, RNK] fp8 packed [128, (j, r)]
    v8_d = nc.dram_tensor("V8", [128, DCH * RNK], f8, kind="ExternalInput")
    # U-factor (U*S scaled): k-pair-packed [128, 2, NN] flat [128, 2*NN] fp8
    su_d = nc.dram_tensor("SU8", [128, 2 * NN], f8, kind="ExternalInput")
    comb_d = nc.dram_tensor("combr", [1, NN], bf16, kind="ExternalInput")
    wcs_d = nc.dram_tensor("WcsT", [128, DCH * NR], bf16, kind="ExternalInput")
    rbias_d = nc.dram_tensor("rbias", [1, NR], bf16, kind="ExternalInput")
    wcg_d = nc.dram_tensor("WcgT", [128, DCH * NR], bf16, kind="ExternalInput")
    gbias_d = nc.dram_tensor("gbias", [1, NR], bf16, kind="ExternalInput")
    out_d = nc.dram_tensor("gatesT", [NR, BSH], f32, kind="ExternalOutput")

    # DRAM bounce buffers for the softmax-tail transpose
    adjx_d = nc.dram_tensor("adjx", [BSH, NN], bf16)
    rawf_d = nc.dram_tensor("rawf", [1, BSH * NR], bf16)

    with tile.TileContext(nc) as tc:
        with (
            tc.tile_pool(name="const", bufs=1) as constp,
            tc.tile_pool(name="xin", bufs=4) as xin,
            tc.tile_pool(name="mid", bufs=1) as mid,
        ):
            # --- constants ---
            ones128 = constp.tile([128, 1], f8)
            nc.gpsimd.memset(ones128[:], 1.0)
            ones4b = constp.tile([1, BSH], bf16)
            nc.gpsimd.memset(ones4b[:], 1.0)
            ones64 = constp.tile([1, NR], bf16)
            nc.gpsimd.memset(ones64[:], 1.0)
            # Preload the Exp ACT table off the critical path.
            scr_in = constp.tile([1, 4], f32)
            nc.gpsimd.memset(scr_in[:], 1.0)
            scr_out = constp.tile([1, 4], f32)
            nc.scalar.activation(scr_out[:], scr_in[:], AF.Exp)

            v8_t = constp.tile([128, DCH * RNK], f8)
            su_t = constp.tile([128, 2 * NN], f8)
            comb_t = constp.tile([1, NN], bf16)
            wcs_t = constp.tile([128, DCH * NR], bf16)
            rbias_t = constp.tile([1, NR], bf16)
            wcg_t = constp.tile([128, DCH * NR], bf16)
            gbias_t = constp.tile([1, NR], bf16)

            # Weights/consts ride the SCALAR queue (x owns sync+gpsimd).
            nc.scalar.dma_start(v8_t[:], v8_d[:])
            nc.scalar.dma_start(wcs_t[:], wcs_d[:])
            nc.scalar.dma_start(comb_t[:], comb_d[:])
            nc.scalar.dma_start(rbias_t[:], rbias_d[:])
            nc.scalar.dma_start(wcg_t[:], wcg_d[:])
            nc.scalar.dma_start(gbias_t[:], gbias_d[:])
            nc.scalar.dma_start(su_t[:], su_d[:])

            # --- phase A: per-sample T-sums, pipelined via single-bank
            # PSUM accumulators (bank b holds sample b's 8 d-chunk sums) ---
            mean_xT = mid.tile([128, DCH * BSH], bf16)
            mean8 = None  # unused name guard
            with tc.tile_pool(name="pmean", bufs=4, space="PSUM") as pmean_p:
                for b in range(BSH):
                    # two banks per sample; consecutive matmuls alternate
                    # banks (pipelining) while each bank hosts only one
                    # OPEN accumulation group at a time (an interleaved
                    # start=True can reset same-bank has_written bits
                    # mid-accumulation)
                    pmean = pmean_p.tile(
                        [128, 2 * 512], f32, name=f"pm{b}", tag="pm"
                    )
                    xt = xin.tile([128, SCH * D], f8, name="xt", tag="xt")
                    eng = nc.sync if (b % 2 == 0) else nc.gpsimd
                    eng.dma_start(
                        xt[:],
                        xs_d[b, :, :].rearrange("(p s) d -> p (s d)", s=SCH),
                    )
                    for g4 in range(4):
                        for s in range(SCH):
                            for jj in range(2):
                                j = 2 * g4 + jj
                                nc.tensor.matmul(
                                    pmean[:, jj * 512 + g4 : jj * 512 + g4 + 1],
                                    xt[
                                        :,
                                        s * D + j * 128 : s * D + (j + 1) * 128,
                                    ],
                                    ones128[:],
                                    start=(s == 0),
                                    stop=(s == SCH - 1),
                                )
                    # slot i of mean_xT holds d-chunk 2*(i%4)+(i//4);
                    # downstream weights are host-packed in the same order
                    for jj in range(2):
                        nc.scalar.activation(
                            mean_xT[:].rearrange("p (s b) -> p s b", b=BSH)[
                                :, jj * 4 : (jj + 1) * 4, b : b + 1
                            ],
                            pmean[:, jj * 512 : jj * 512 + 4].unsqueeze(2),
                            AF.Copy,
                            bias=0.0,
                            scale=1.0 / T,
                        )

            # --- warp stage 1: tT = (VS*V).T @ mean, directly transposed
            # [RNK on partitions, 4]; fp8 copy (x TS) for stage 2 ---
            t8 = mid.tile([128, 2 * 16], f8)
            raw_bf = mid.tile([BSH, NR], bf16)
            rawT_rep = mid.tile([NR, BSH * NR], bf16)
            adj_expT = mid.tile([NR, BSH * NR], bf16)
            raw_flat = mid.tile([1, BSH * NR], bf16)
            ssumT = mid.tile([NR, BSH], f32)
            qsumT = mid.tile([NR, BSH], f32)
            qprodT = mid.tile([NR, BSH * NR], bf16)
            with tc.tile_pool(name="pc", bufs=1, space="PSUM") as pc_p:
                ptT = pc_p.tile([128, 2 * BSH], f32, tag="ptT")
                praw = pc_p.tile([BSH, NR], f32, tag="praw")
                prep = pc_p.tile([NR, BSH * NR], f32, tag="prep")
                pgT = pc_p.tile([NR, BSH], f32, tag="pgT")
                for rc in range(2):
                    for j in range(DCH):
                        nc.tensor.matmul(
                            ptT[:, rc * BSH : (rc + 1) * BSH],
                            v8_t[:].rearrange("p (j r) -> p j r", r=RNK)[
                                :, j, rc * 128 : (rc + 1) * 128
                            ],
                            mean_xT[:, j * BSH : (j + 1) * BSH],
                            start=(j == 0),
                            stop=(j == DCH - 1),
                        )
                nc.scalar.activation(
                    t8[:].rearrange("p (k w) -> p k w", w=16)[:, :, 0:BSH],
                    ptT[:].rearrange("p (k b) -> p k b", b=BSH),
                    AF.Copy,
                    bias=0.0,
                    scale=TS,
                )

                # raw = mean @ WcsT + rbias [4, 64], then replicated to
                # [64, (b, n)] via a DRAM bounce + rank-1
                for j in range(DCH):
                    nc.tensor.matmul(
                        praw[:],
                        mean_xT[:, j * BSH : (j + 1) * BSH],
                        wcs_t[:, j * NR : (j + 1) * NR],
                        start=(j == 0),
                        stop=False,
                    )
                nc.tensor.matmul(
                    praw[:], ones4b[:], rbias_t[:], start=False, stop=True
                )
                nc.scalar.copy(raw_bf[:], praw[:])
                nc.scalar.dma_start(
                    rawf_d[:, :].rearrange("o (b n) -> (o b) n", b=BSH),
                    raw_bf[:],
                )
                nc.scalar.dma_start(raw_flat[:], rawf_d[:, :])
                nc.tensor.matmul(
                    prep[:], ones64[:], raw_flat[:], start=True, stop=True
                )
                nc.scalar.copy(rawT_rep[:], prep[:])

                # --- warp stage 2 in eighths: comb row DVE-preloaded into
                # PSUM (f32), one fp8 DoubleRow matmul on top, exp out ---
                adj_exp = mid.tile([BSH, NN], bf16)
                with tc.tile_pool(name="pw", bufs=3, space="PSUM") as pw_p:
                    for e in range(8):
                        pw = pw_p.tile([BSH, 512], f32, name="pw", tag="pw")
                        # comb via rank-1 matmul: only TensorE writes set the
                        # PSUM has_written bits that accumulation needs
                        nc.tensor.matmul(
                            pw[:],
                            ones4b[:],
                            comb_t[:, e * 512 : (e + 1) * 512],
                            start=True,
                            stop=False,
                        )
                        nc.tensor.matmul(
                            pw[:],
                            t8[:].rearrange("p (k w) -> p k w", w=16)[
                                :, :, 0:BSH
                            ],
                            su_t[:].rearrange("p (k n) -> p k n", k=2)[
                                :, :, e * 512 : (e + 1) * 512
                            ],
                            start=False,
                            stop=True,
                            perf_mode=DR,
                        )
                        ae = adj_exp[:, e * 512 : (e + 1) * 512]
                        nc.scalar.activation(
                            ae, pw[:], AF.Exp, bias=0.0, scale=0.1 / CSC
                        )
                        # bounce through DRAM into [m, (b, n)], per half
                        if e % 4 == 3:
                            hf = e // 4
                            eng = nc.sync if hf == 0 else nc.gpsimd
                            sl = slice(hf * 2048, (hf + 1) * 2048)
                            eng.dma_start(adjx_d[:, sl], adj_exp[:, sl])
                            eng.dma_start(
                                adj_expT[hf * 32 : (hf + 1) * 32, :].rearrange(
                                    "m (b n) -> m b n", n=NR
                                ),
                                adjx_d[:, sl].rearrange(
                                    "b (m n) -> m b n", n=NR
                                ),
                            )

                # gate projection, directly transposed: pgT[m, b]
                for j in range(DCH):
                    nc.tensor.matmul(
                        pgT[:],
                        wcg_t[:, j * NR : (j + 1) * NR],
                        mean_xT[:, j * BSH : (j + 1) * BSH],
                        start=(j == 0),
                        stop=False,
                    )
                nc.tensor.matmul(
                    pgT[:], gbias_t[:], ones4b[:], start=False, stop=True
                )

                # --- softmax tail in halves of m (32 partitions each) ---
                for hf in range(2):
                    sl = slice(hf * 32, (hf + 1) * 32)
                    aeg = adj_expT[sl, :].rearrange("m (b n) -> m b n", n=NR)
                    nc.vector.reduce_sum(
                        ssumT[sl, :], aeg, axis=mybir.AxisListType.X
                    )
                    qp = qprodT[sl, :].rearrange("m (b n) -> m b n", n=NR)
                    nc.vector.tensor_mul(
                        qp,
                        aeg,
                        rawT_rep[sl, :].rearrange("m (b n) -> m b n", n=NR),
                    )
                    nc.vector.reduce_sum(
                        qsumT[sl, :], qp, axis=mybir.AxisListType.X
                    )

                # --- finals (transposed): gatesT = sigmoid((gT+qsum/ssum)/2)
                rinvT = mid.tile([NR, BSH], f32)
                nc.vector.reciprocal(rinvT[:], ssumT[:])
                extraT = mid.tile([NR, BSH], f32)
                nc.vector.tensor_mul(extraT[:], qsumT[:], rinvT[:])
                logitsT = mid.tile([NR, BSH], f32)
                nc.vector.tensor_add(logitsT[:], pgT[:], extraT[:])
                enT = mid.tile([NR, BSH], f32)
                nc.scalar.activation(
                    enT[:], logitsT[:], AF.Exp, bias=0.0, scale=-0.5
                )
                ep1T = mid.tile([NR, BSH], f32)
                nc.vector.tensor_scalar_add(ep1T[:], enT[:], 1.0)
                gatesT_s = mid.tile([NR, BSH], f32)
                nc.vector.reciprocal(gatesT_s[:], ep1T[:])
                nc.sync.dma_start(out_d[:], gatesT_s[:])

    nc.compile()
    return nc


def _get_nc():
    if "nc" not in _cache:
        _cache["nc"] = _build_nc()
    return _cache["nc"]


def _prep_weights(Wc, bc, Wg, bg, Ww, bw, adjacency, summaries):
    import ml_dtypes

    bf16 = ml_dtypes.bfloat16
    f8 = ml_dtypes.float8_e4m3
    f32 = np.float32

    # phase A writes d-chunk 2*(i%4)+(i//4) into mean slot i; pack all
    # d-chunked weights in the same slot order
    PERM = [2 * (i % 4) + (i // 4) for i in range(DCH)]
    Wc_f = np.asarray(Wc, dtype=f32)
    bc_f = np.asarray(bc, dtype=f32)
    Ww_f = np.asarray(Ww, dtype=f32)
    bw_f = np.asarray(bw, dtype=f32).reshape(NN)
    adj_f = np.asarray(adjacency, dtype=f32).reshape(NN)
    sm_f = np.asarray(summaries, dtype=f32)
    Wg_f = np.asarray(Wg, dtype=f32)
    bg_f = np.asarray(bg, dtype=f32)

    # Fold Wc into the warp projection and factor to rank RNK.
    Wcw = Ww_f @ Wc_f  # [NN, D]
    U, S, Vt = np.linalg.svd(Wcw, full_matrices=False)
    Vr = Vt[:RNK].T  # [D, RNK]
    USr = U[:, :RNK] * S[:RNK]  # [NN, RNK]
    V8 = np.ascontiguousarray(
        (VS * Vr).reshape(DCH, 128, RNK)[PERM].transpose(1, 0, 2)
        .reshape(128, DCH * RNK).astype(f8)
    )
    # SU pair-packed: [RNK, NN] -> [2 k-sub, 128, NN] -> [128, 2, NN]
    SU8 = np.ascontiguousarray(
        (US * USr).T.reshape(2, 128, NN).transpose(1, 0, 2)
        .reshape(128, 2 * NN).astype(f8)
    )
    comb = CSC * (10.0 * adj_f + bw_f + Ww_f @ bc_f)
    comb_r = np.ascontiguousarray(comb.reshape(1, NN).astype(bf16))

    Wcs = sm_f @ Wc_f
    WcsT_p = np.ascontiguousarray(
        Wcs.T.reshape(DCH, 128, NR)[PERM].transpose(1, 0, 2)
        .reshape(128, DCH * NR).astype(bf16)
    )
    rbias = np.ascontiguousarray((sm_f @ bc_f).reshape(1, NR).astype(bf16))

    Wcg = Wg_f @ Wc_f
    WcgT_p = np.ascontiguousarray(
        Wcg.T.reshape(DCH, 128, NR)[PERM].transpose(1, 0, 2)
        .reshape(128, DCH * NR).astype(bf16)
    )
    gbias = np.ascontiguousarray(
        (bg_f + Wg_f @ bc_f).reshape(1, NR).astype(bf16)
    )
    return {
        "V8": V8,
        "SU8": SU8,
        "combr": comb_r,
        "WcsT": WcsT_p,
        "rbias": rbias,
        "WcgT": WcgT_p,
        "gbias": gbias,
    }


def _make_in_maps(x, summaries, Wc, bc, Wg, bg, Ww, bw, adjacency):
    import ml_dtypes

    f8 = ml_dtypes.float8_e4m3
    x8 = np.ascontiguousarray(np.asarray(x, dtype=np.float32).astype(f8))
    w = _prep_weights(Wc, bc, Wg, bg, Ww, bw, adjacency, summaries)
    in_maps = []
    for c in range(N_CORES):
        m = {"xs": np.ascontiguousarray(x8[c * BSH : (c + 1) * BSH])}
        m.update(w)
        in_maps.append(m)
    return in_maps


def run_kernel_raw(trace=False, **inputs):
    """Returns (gates [32, 64], BassKernelResults)."""
    from concourse.bass_utils import run_bass_kernel_spmd

    nc = _get_nc()
    in_maps = _make_in_maps(**inputs)
    res = run_bass_kernel_spmd(nc, in_maps, list(range(N_CORES)), trace=trace)
    gates = np.concatenate(
        [np.asarray(res.results[c]["gatesT"]).T for c in range(N_CORES)], axis=0
    ).astype(np.float32)
    return gates, res


def kernel(**inputs):
    gates, _ = run_kernel_raw(trace=False, **inputs)
    return gates


# revision 3
# speedup vs baseline: 1.0453x; 1.0453x over previous
"""Trainium2 Bass kernel for nn_MindPalaceRouter (v6).

Computation (reference):
    ctx  = mean_T(x) @ Wc.T + bc                      [B, d]
    warp = (ctx @ Ww.T + bw).reshape(B, n, n) * 0.1
    adj  = softmax(adjacency + warp, axis=-1)
    raw  = ctx @ summaries.T                          [B, n]
    gate = sigmoid((ctx @ Wg.T + bg + adj @ raw) / 2) [B, n]

Strategy: data-parallel over B (4 samples/core, no collectives).  Wc is
folded into every downstream projection on the host (weights-only
reparametrization), so ctx is never materialized:
    warp path:  Wcw = Ww @ Wc, factored by SVD to rank 256 (keeps
                absmax-rel error ~3e-5, three orders under the 2e-2
                gate) and run as two thin fp8 matmuls, the second in
                DoubleRow mode.
    raw  path:  Wcs = summaries @ Wc  (bf16)
    gate path:  Wcg = Wg @ Wc         (bf16, computed transposed)
x ships fp8 with 16KB-contiguous per-partition runs; the T-sums
pipeline across samples in per-sample single-bank PSUM accumulators.
The adjacency+bias row is preloaded into each warp PSUM tile by the
(otherwise idle) DVE in f32.  The softmax message tail runs in a
transposed [m, (b, n)] layout reached via a DRAM bounce, so its
reduces are 32-64-partition-wide; the finals stay transposed and the
host transposes the [64, 4] per-core output back.
"""

import sys

if "/opt/trn_rl_repo" not in sys.path:
    sys.path.insert(0, "/opt/trn_rl_repo")

import numpy as np

N_CORES = 8
B, T, D, NR = 32, 2048, 1024, 64
NN = NR * NR  # 4096
BSH = B // N_CORES  # 4 samples per core
DCH = D // 128  # 8 d-chunks
SCH = 16  # T-rows per partition per x tile (16KB contiguous runs)
RNK = 256  # Wcw SVD truncation rank
VS = 64.0  # fp8 V-factor pre-scale (V entries ~1/32)
TS = 2.0  # extra scale on the fp8 intermediate copy
US = 64.0  # fp8 U-factor pre-scale (U*S entries ~1/64)
CSC = VS * TS * US  # 8192: total scale compensated in the Exp

_cache = {}


def _build_nc():
    import concourse.bass as bass
    import concourse.tile as tile
    from concourse import bacc, mybir

    f32 = mybir.dt.float32
    bf16 = mybir.dt.bfloat16
    f8 = mybir.dt.float8e4
    AF = mybir.ActivationFunctionType
    DR = mybir.MatmulPerfMode.DoubleRow

    nc = bacc.Bacc(
        "TRN2",
        target_bir_lowering=False,
        debug=False,
        enable_asserts=True,
        num_devices=N_CORES,
    )

    xs_d = nc.dram_tensor("xs", [BSH, T, D], f8, kind="ExternalInput")
    # V-factor: [d, RNK] fp8 packed [128, (j, r)]
    v8_d = nc.dram_tensor("V8", [128, DCH * RNK], f8, kind="ExternalInput")
    # U-factor (U*S scaled): k-pair-packed [128, 2, NN] flat [128, 2*NN] fp8
    su_d = nc.dram_tensor("SU8", [128, 2 * NN], f8, kind="ExternalInput")
    comb_d = nc.dram_tensor("combr", [1, NN], bf16, kind="ExternalInput")
    wcs_d = nc.dram_tensor("WcsT", [128, DCH * NR], bf16, kind="ExternalInput")
    rbias_d = nc.dram_tensor("rbias", [1, NR], bf16, kind="ExternalInput")
    wcg_d = nc.dram_tensor("WcgT", [128, DCH * NR], bf16, kind="ExternalInput")
    gbias_d = nc.dram_tensor("gbias", [1, NR], bf16, kind="ExternalInput")
    out_d = nc.dram_tensor("gatesT", [NR, BSH], f32, kind="ExternalOutput")

    # DRAM bounce buffers for the softmax-tail transpose
    adjx_d = nc.dram_tensor("adjx", [BSH, NN], bf16)
    rawf_d = nc.dram_tensor("rawf", [1, BSH * NR], bf16)

    with tile.TileContext(nc) as tc:
        with (
            tc.tile_pool(name="const", bufs=1) as constp,
            tc.tile_pool(name="xin", bufs=4) as xin,
            tc.tile_pool(name="mid", bufs=1) as mid,
        ):
            # --- constants ---
            ones128 = constp.tile([128, 1], f8)
            nc.gpsimd.memset(ones128[:], 1.0)
            ones4b = constp.tile([1, BSH], bf16)
            nc.gpsimd.memset(ones4b[:], 1.0)
            ones64 = constp.tile([1, NR], bf16)
            nc.gpsimd.memset(ones64[:], 1.0)
            # Preload the Exp ACT table off the critical path.
            scr_in = constp.tile([1, 4], f32)
            nc.gpsimd.memset(scr_in[:], 1.0)
            scr_out = constp.tile([1, 4], f32)
            nc.scalar.activation(scr_out[:], scr_in[:], AF.Exp)

            v8_t = constp.tile([128, DCH * RNK], f8)
            su_t = constp.tile([128, 2 * NN], f8)
            comb_t = constp.tile([1, NN], bf16)
            wcs_t = constp.tile([128, DCH * NR], bf16)
            rbias_t = constp.tile([1, NR], bf16)
            wcg_t = constp.tile([128, DCH * NR], bf16)
            gbias_t = constp.tile([1, NR], bf16)

            # Weights/consts ride the SCALAR queue (x owns sync+gpsimd).
            nc.scalar.dma_start(v8_t[:], v8_d[:])
            nc.scalar.dma_start(wcs_t[:], wcs_d[:])
            nc.scalar.dma_start(comb_t[:], comb_d[:])
            nc.scalar.dma_start(rbias_t[:], rbias_d[:])
            nc.scalar.dma_start(wcg_t[:], wcg_d[:])
            nc.scalar.dma_start(gbias_t[:], gbias_d[:])
            nc.scalar.dma_start(su_t[:], su_d[:])

            # --- phase A: per-sample T-sums, pipelined via single-bank
            # PSUM accumulators (bank b holds sample b's 8 d-chunk sums) ---
            mean_xT = mid.tile([128, DCH * BSH], bf16)
            with tc.tile_pool(name="pmean", bufs=4, space="PSUM") as pmean_p:
                for b in range(BSH):
                    # two banks per sample; consecutive matmuls alternate
                    # banks (pipelining) while each bank hosts only one
                    # OPEN accumulation group at a time (an interleaved
                    # start=True can reset same-bank has_written bits
                    # mid-accumulation)
                    pmean = pmean_p.tile(
                        [128, 2 * 512], f32, name=f"pm{b}", tag="pm"
                    )
                    xt = xin.tile([128, SCH * D], f8, name="xt", tag="xt")
                    eng = nc.sync if (b % 2 == 0) else nc.gpsimd
                    eng.dma_start(
                        xt[:],
                        xs_d[b, :, :].rearrange("(p s) d -> p (s d)", s=SCH),
                    )
                    for g4 in range(4):
                        for s in range(SCH):
                            for jj in range(2):
                                j = 2 * g4 + jj
                                nc.tensor.matmul(
                                    pmean[:, jj * 512 + g4 : jj * 512 + g4 + 1],
                                    xt[
                                        :,
                                        s * D + j * 128 : s * D + (j + 1) * 128,
                                    ],
                                    ones128[:],
                                    start=(s == 0),
                                    stop=(s == SCH - 1),
                                )
                    # slot i of mean_xT holds d-chunk 2*(i%4)+(i//4);
                    # downstream weights are host-packed in the same order
                    for jj in range(2):
                        nc.scalar.activation(
                            mean_xT[:].rearrange("p (s b) -> p s b", b=BSH)[
                                :, jj * 4 : (jj + 1) * 4, b : b + 1
                            ],
                            pmean[:, jj * 512 : jj * 512 + 4].unsqueeze(2),
                            AF.Copy,
                            bias=0.0,
                            scale=1.0 / T,
                        )

            # --- warp stage 1: tT = (VS*V).T @ mean, directly transposed
            # [RNK on partitions, 4]; fp8 copy (x TS) for stage 2 ---
            t8 = mid.tile([128, 2 * 16], f8)
            raw_bf = mid.tile([BSH, NR], bf16)
            rawT_rep = mid.tile([NR, BSH * NR], bf16)
            adj_expT = mid.tile([NR, BSH * NR], bf16)
            raw_flat = mid.tile([1, BSH * NR], bf16)
            ssumT = mid.tile([NR, BSH], f32)
            qsumT = mid.tile([NR, BSH], f32)
            qprodT = mid.tile([NR, BSH * NR], bf16)
            with tc.tile_pool(name="pc", bufs=1, space="PSUM") as pc_p:
                ptT = pc_p.tile([128, 2 * BSH], f32, tag="ptT")
                praw = pc_p.tile([BSH, NR], f32, tag="praw")
                prep = pc_p.tile([NR, BSH * NR], f32, tag="prep")
                pgT = pc_p.tile([NR, BSH], f32, tag="pgT")
                for rc in range(2):
                    for j in range(DCH):
                        nc.tensor.matmul(
                            ptT[:, rc * BSH : (rc + 1) * BSH],
                            v8_t[:].rearrange("p (j r) -> p j r", r=RNK)[
                                :, j, rc * 128 : (rc + 1) * 128
                            ],
                            mean_xT[:, j * BSH : (j + 1) * BSH],
                            start=(j == 0),
                            stop=(j == DCH - 1),
                        )
                nc.scalar.activation(
                    t8[:].rearrange("p (k w) -> p k w", w=16)[:, :, 0:BSH],
                    ptT[:].rearrange("p (k b) -> p k b", b=BSH),
                    AF.Copy,
                    bias=0.0,
                    scale=TS,
                )

                # raw = mean @ WcsT + rbias [4, 64], then replicated to
                # [64, (b, n)] via a DRAM bounce + rank-1
                for j in range(DCH):
                    nc.tensor.matmul(
                        praw[:],
                        mean_xT[:, j * BSH : (j + 1) * BSH],
                        wcs_t[:, j * NR : (j + 1) * NR],
                        start=(j == 0),
                        stop=False,
                    )
                nc.tensor.matmul(
                    praw[:], ones4b[:], rbias_t[:], start=False, stop=True
                )
                nc.scalar.copy(raw_bf[:], praw[:])
                nc.scalar.dma_start(
                    rawf_d[:, :].rearrange("o (b n) -> (o b) n", b=BSH),
                    raw_bf[:],
                )
                nc.scalar.dma_start(raw_flat[:], rawf_d[:, :])
                nc.tensor.matmul(
                    prep[:], ones64[:], raw_flat[:], start=True, stop=True
                )
                nc.scalar.copy(rawT_rep[:], prep[:])

                # --- warp stage 2 in eighths: comb row DVE-preloaded into
                # PSUM (f32), one fp8 DoubleRow matmul on top, exp out ---
                adj_exp = mid.tile([BSH, NN], bf16)
                with tc.tile_pool(name="pw", bufs=3, space="PSUM") as pw_p:
                    for e in range(8):
                        pw = pw_p.tile([BSH, 512], f32, name="pw", tag="pw")
                        # comb via rank-1 matmul: only TensorE writes set the
                        # PSUM has_written bits that accumulation needs
                        nc.tensor.matmul(
                            pw[:],
                            ones4b[:],
                            comb_t[:, e * 512 : (e + 1) * 512],
                            start=True,
                            stop=False,
                        )
                        nc.tensor.matmul(
                            pw[:],
                            t8[:].rearrange("p (k w) -> p k w", w=16)[
                                :, :, 0:BSH
                            ],
                            su_t[:].rearrange("p (k n) -> p k n", k=2)[
                                :, :, e * 512 : (e + 1) * 512
                            ],
                            start=False,
                            stop=True,
                            perf_mode=DR,
                        )
                        ae = adj_exp[:, e * 512 : (e + 1) * 512]
                        nc.scalar.activation(
                            ae, pw[:], AF.Exp, bias=0.0, scale=0.1 / CSC
                        )
                        # bounce through DRAM into [m, (b, n)], per half
                        if e % 4 == 3:
                            hf = e // 4
                            eng = nc.sync if hf == 0 else nc.gpsimd
                            sl = slice(hf * 2048, (hf + 1) * 2048)
                            eng.dma_start(adjx_d[:, sl], adj_exp[:, sl])
                            eng.dma_start(
                                adj_expT[hf * 32 : (hf + 1) * 32, :].rearrange(
                                    "m (b n) -> m b n", n=NR
                                ),
                                adjx_d[:, sl].rearrange(
                                    "b (m n) -> m b n", n=NR
                                ),
                            )

                # gate projection, directly transposed: pgT[m, b]
                for j in range(DCH):
                    nc.tensor.matmul(
                        pgT[:],
                        wcg_t[:, j * NR : (j + 1) * NR],
                        mean_xT[:, j * BSH : (j + 1) * BSH],
                        start=(j == 0),
                        stop=False,
                    )
                nc.tensor.matmul(
                    pgT[:], gbias_t[:], ones4b[:], start=False, stop=True
                )

                # --- softmax tail in halves of m (32 partitions each) ---
                for hf in range(2):
                    sl = slice(hf * 32, (hf + 1) * 32)
                    aeg = adj_expT[sl, :].rearrange("m (b n) -> m b n", n=NR)
                    nc.vector.reduce_sum(
                        ssumT[sl, :], aeg, axis=mybir.AxisListType.X
                    )
                    qp = qprodT[sl, :].rearrange("m (b n) -> m b n", n=NR)
                    nc.vector.tensor_mul(
                        qp,
                        aeg,
                        rawT_rep[sl, :].rearrange("m (b n) -> m b n", n=NR),
                    )
                    nc.vector.reduce_sum(
                        qsumT[sl, :], qp, axis=mybir.AxisListType.X
                    )

                # --- finals (transposed): gatesT = sigmoid((gT+qsum/ssum)/2)
                rinvT = mid.tile([NR, BSH], f32)
                nc.vector.reciprocal(rinvT[:], ssumT[:])
                extraT = mid.tile([NR, BSH], f32)
                nc.vector.tensor_mul(extraT[:], qsumT[:], rinvT[:])
                logitsT = mid.tile([NR, BSH], f32)
                nc.vector.tensor_add(logitsT[:], pgT[:], extraT[:])
                enT = mid.tile([NR, BSH], f32)
                nc.scalar.activation(
                    enT[:], logitsT[:], AF.Exp, bias=0.0, scale=-0.5
                )
                ep1T = mid.tile([NR, BSH], f32)
                nc.vector.tensor_scalar_add(ep1T[:], enT[:], 1.0)
                gatesT_s = mid.tile([NR, BSH], f32)
                nc.vector.reciprocal(gatesT_s[:], ep1T[:])
                nc.sync.dma_start(out_d[:], gatesT_s[:])

    nc.compile()
    return nc


def _get_nc():
    if "nc" not in _cache:
        _cache["nc"] = _build_nc()
    return _cache["nc"]


def _prep_weights(Wc, bc, Wg, bg, Ww, bw, adjacency, summaries):
    import ml_dtypes

    bf16 = ml_dtypes.bfloat16
    f8 = ml_dtypes.float8_e4m3
    f32 = np.float32

    # phase A writes d-chunk 2*(i%4)+(i//4) into mean slot i; pack all
    # d-chunked weights in the same slot order
    PERM = [2 * (i % 4) + (i // 4) for i in range(DCH)]
    Wc_f = np.asarray(Wc, dtype=f32)
    bc_f = np.asarray(bc, dtype=f32)
    Ww_f = np.asarray(Ww, dtype=f32)
    bw_f = np.asarray(bw, dtype=f32).reshape(NN)
    adj_f = np.asarray(adjacency, dtype=f32).reshape(NN)
    sm_f = np.asarray(summaries, dtype=f32)
    Wg_f = np.asarray(Wg, dtype=f32)
    bg_f = np.asarray(bg, dtype=f32)

    # Fold Wc into the warp projection and factor to rank RNK.
    Wcw = Ww_f @ Wc_f  # [NN, D]
    U, S, Vt = np.linalg.svd(Wcw, full_matrices=False)
    Vr = Vt[:RNK].T  # [D, RNK]
    USr = U[:, :RNK] * S[:RNK]  # [NN, RNK]
    V8 = np.ascontiguousarray(
        (VS * Vr).reshape(DCH, 128, RNK)[PERM].transpose(1, 0, 2)
        .reshape(128, DCH * RNK).astype(f8)
    )
    # SU pair-packed: [RNK, NN] -> [2 k-sub, 128, NN] -> [128, 2, NN]
    SU8 = np.ascontiguousarray(
        (US * USr).T.reshape(2, 128, NN).transpose(1, 0, 2)
        .reshape(128, 2 * NN).astype(f8)
    )
    comb = CSC * (10.0 * adj_f + bw_f + Ww_f @ bc_f)
    comb_r = np.ascontiguousarray(comb.reshape(1, NN).astype(bf16))

    Wcs = sm_f @ Wc_f
    WcsT_p = np.ascontiguousarray(
        Wcs.T.reshape(DCH, 128, NR)[PERM].transpose(1, 0, 2)
        .reshape(128, DCH * NR).astype(bf16)
    )
    rbias = np.ascontiguousarray((sm_f @ bc_f).reshape(1, NR).astype(bf16))

    Wcg = Wg_f @ Wc_f
    WcgT_p = np.ascontiguousarray(
        Wcg.T.reshape(DCH, 128, NR)[PERM].transpose(1, 0, 2)
        .reshape(128, DCH * NR).astype(bf16)
    )
    gbias = np.ascontiguousarray(
        (bg_f + Wg_f @ bc_f).reshape(1, NR).astype(bf16)
    )
    return {
        "V8": V8,
        "SU8": SU8,
        "combr": comb_r,
        "WcsT": WcsT_p,
        "rbias": rbias,
        "WcgT": WcgT_p,
        "gbias": gbias,
    }


def _make_in_maps(x, summaries, Wc, bc, Wg, bg, Ww, bw, adjacency):
    import ml_dtypes

    f8 = ml_dtypes.float8_e4m3
    x8 = np.ascontiguousarray(np.asarray(x, dtype=np.float32).astype(f8))
    w = _prep_weights(Wc, bc, Wg, bg, Ww, bw, adjacency, summaries)
    in_maps = []
    for c in range(N_CORES):
        m = {"xs": np.ascontiguousarray(x8[c * BSH : (c + 1) * BSH])}
        m.update(w)
        in_maps.append(m)
    return in_maps


def run_kernel_raw(trace=False, **inputs):
    """Returns (gates [32, 64], BassKernelResults)."""
    from concourse.bass_utils import run_bass_kernel_spmd

    nc = _get_nc()
    in_maps = _make_in_maps(**inputs)
    res = run_bass_kernel_spmd(nc, in_maps, list(range(N_CORES)), trace=trace)
    gates = np.concatenate(
        [np.asarray(res.results[c]["gatesT"]).T for c in range(N_CORES)], axis=0
    ).astype(np.float32)
    return gates, res


def kernel(**inputs):
    gates, _ = run_kernel_raw(trace=False, **inputs)
    return gates
